# revision 5
# baseline (speedup 1.0000x reference)
"""Single-head causal self-attention on 8 Trainium2 NeuronCores.

Reference computation (per batch b):
    k = x @ Wk.T ; q = x @ Wq.T ; v = x @ Wv.T
    wei = softmax(mask(q @ k.T / sqrt(H)))
    out = wei @ v

Strategy:
  - Data parallel: B=256 sharded across 8 cores (32 batches each), weights
    replicated. No cross-core communication.
  - Algebraic fusion: q @ k.T = x (Wq.T Wk) x.T. G = Wq.T @ Wk * scale is
    precomputed on the host, which halves the per-batch q/k projection work.
  - All per-batch operands are pre-laid-out on the host so the device never
    transposes: x is shipped as xT[b] = x[b].T (contraction dim on
    partitions), Wv as WvT = Wv.T.
  - bf16 operands (f32 PSUM accumulation): 1 cycle/row matmuls at any free
    size, half the DMA bytes, ~4.5e-3 rel err vs the 2e-2 gate.
  - Batches are processed in PAIRS: one input DMA and one output DMA per
    pair (the sync engine pays a fixed ~650ns per DMA regardless of size),
    and the z2 = G @ x.T matmuls of the two batches fuse into N=512
    instructions.
  - Scores are computed in transposed layout ST[s, t] so exp(ST) feeds the
    output matmuls as the stationary operand. Per batch the score PSUM is
    laid out [s0,t-lo | s1,t-hi | s0,t-hi] so ONE exp activation covers all
    three blocks and ONE mask multiply covers the two causal-triangle
    blocks. Causally-dead blocks (s1, t-lo) are never computed.
  - Softmax denominator: V is augmented with ones columns so the output
    matmul also yields r[t] = sum_s exp(ST[s,t]); normalization is a
    reciprocal + per-partition scale.
  - Software-pipelined schedule (A: load+projections, B1: scores+exp+mask,
    B2: output+normalize+store) with a 2-pair skew so exp/mask latency hides
    under the next pair's projection matmuls; elementwise work is spread
    over ACT/DVE/GpSimd so PE is the only near-saturated engine.
  - One merged SBUF tile per kind per pair (z2/v/e/rec) to minimize
    tile-framework semaphore count: each semaphore costs ~110ns to reset in
    the end-of-kernel teardown ladder.
"""

import ml_dtypes
import numpy as np

import concourse.bass as bass
import concourse.mybir as mybir
from concourse import bacc
import concourse.tile as tile
from concourse.bass_utils import run_bass_kernel_spmd

B, T, C, H = 256, 256, 384, 384
NCORES = 8
NB = B // NCORES  # batches per core
P = 128
CC = C // P  # 3 chunks of the embedding dim
SCALE = float(H) ** -0.5
F32 = mybir.dt.float32
BF16 = mybir.dt.bfloat16
NPBF16 = ml_dtypes.bfloat16
VW = H + 8  # v block augmented with 8 ones columns (16B-aligned in bf16)
XW = CC * 2 * T  # paired input tile width: col = j*512 + i*256 + t
OW = 2 * 2 * H  # paired output tile width: col = i*768 + tc*384 + h
VBUFS = 3  # ring depth of the merged v tile (ones written on first pass)


def build_bass(nb: int = NB):
    assert nb % 2 == 0
    nb2 = nb // 2
    nc = bacc.Bacc(
        "TRN2",
        target_bir_lowering=False,
        debug=False,
        enable_asserts=False,
        num_devices=NCORES,
    )
    xt_d = nc.dram_tensor("xt", [nb2, P, XW], BF16, kind="ExternalInput").ap()
    g_d = nc.dram_tensor("G", [C, C], BF16, kind="ExternalInput").ap()
    wvT_d = nc.dram_tensor("WvT", [C, H], BF16, kind="ExternalInput").ap()
    m_d = nc.dram_tensor("M2", [P, 2 * P], BF16, kind="ExternalInput").ap()
    out_d = nc.dram_tensor("out", [nb2, P, OW], BF16, kind="ExternalOutput").ap()

    EXP = mybir.ActivationFunctionType.Exp
    CPY = mybir.ActivationFunctionType.Copy

    with tile.TileContext(nc) as tc:
        with (
            tc.tile_pool(name="const", bufs=1) as cpool,
            tc.tile_pool(name="io", bufs=3) as iop,
            tc.tile_pool(name="wk", bufs=3) as wkp,
            tc.tile_pool(name="ps", bufs=7, space="PSUM") as psp,
        ):
            # g chunks first on the sync queue (the first z2 matmul needs
            # them + xt[0]); wvT/M2 issue in parallel from the gpsimd queue
            g_s = []
            for i in range(CC):
                gt = cpool.tile([P, C], BF16, name=f"g{i}")
                nc.sync.dma_start(gt, g_d[i * P : (i + 1) * P, :])
                g_s.append(gt)
            wvT_s = []
            for i in range(CC):
                wt = cpool.tile([P, H], BF16, name=f"wvT{i}")
                nc.gpsimd.dma_start(wt, wvT_d[i * P : (i + 1) * P, :])
                wvT_s.append(wt)
            m2 = cpool.tile([P, 2 * P], BF16, name="m2")
            nc.gpsimd.dma_start(m2, m_d)

            # per-pair state threaded between pipeline stages
            st_xt = [None] * nb2
            st_z2 = [None] * nb2
            st_v = [None] * nb2
            st_e = [None] * nb2

            def xsl(xt, i, j, lo, hi):
                # lhsT slice of batch i, embed chunk j, seq cols [lo:hi)
                base = j * 2 * T + i * T
                return xt[:, base + lo : base + hi]

            def stage_a(pb):
                # paired load + fused z2 matmuls + per-batch v projections
                xt = iop.tile([P, XW], BF16, name="xt", tag="xt")
                nc.sync.dma_start(xt, xt_d[pb])
                st_xt[pb] = xt

                z2 = wkp.tile([P, CC * 512], BF16, name="z2", tag="z2")
                for c2 in range(CC):
                    pz = psp.tile([P, 512], F32, name="pz", tag="pm")
                    for c1 in range(CC):
                        nc.tensor.matmul(
                            pz,
                            lhsT=g_s[c1][:, c2 * P : (c2 + 1) * P],
                            rhs=xt[:, c1 * 512 : (c1 + 1) * 512],
                            start=(c1 == 0),
                            stop=(c1 == CC - 1),
                        )
                    nc.scalar.activation(z2[:, c2 * 512 : (c2 + 1) * 512], pz, CPY)
                st_z2[pb] = z2

                vau = wkp.tile([P, 4 * VW], BF16, name="v", tag="v", bufs=VBUFS)
                if pb < VBUFS:
                    for q in range(4):
                        nc.gpsimd.memset(vau[:, q * VW + H : (q + 1) * VW], 1.0)
                for i in range(2):
                    for sc in range(2):
                        pv = psp.tile([P, 512], F32, name="pv", tag="pm")[:, :H]
                        for j in range(CC):
                            nc.tensor.matmul(
                                pv,
                                lhsT=xsl(xt, i, j, sc * P, (sc + 1) * P),
                                rhs=wvT_s[j],
                                start=(j == 0),
                                stop=(j == CC - 1),
                            )
                        q = 2 * i + sc
                        dst = vau[:, q * VW : q * VW + H]
                        if sc == 0:
                            nc.scalar.activation(dst, pv, CPY)
                        else:
                            nc.vector.tensor_copy(dst, pv)
                st_v[pb] = vau

            def stage_b1(pb):
                # scores (transposed) + single exp + single causal mask per
                # batch; psum layout [s0,t-lo | s1,t-hi | s0,t-hi]
                xt, z2 = st_xt[pb], st_z2[pb]
                es = []
                for i in range(2):
                    pst = psp.tile([P, 512], F32, name="pst", tag="pm")
                    zlo = lambda j: z2[:, j * 512 + i * T : j * 512 + i * T + P]
                    zhi = lambda j: z2[:, j * 512 + i * T + P : j * 512 + (i + 1) * T]
                    for j in range(CC):
                        nc.tensor.matmul(
                            pst[:, 0:P],
                            lhsT=xsl(xt, i, j, 0, P),
                            rhs=zlo(j),
                            start=(j == 0),
                            stop=(j == CC - 1),
                        )
                    for j in range(CC):
                        nc.tensor.matmul(
                            pst[:, P : 2 * P],
                            lhsT=xsl(xt, i, j, P, 2 * P),
                            rhs=zhi(j),
                            start=(j == 0),
                            stop=(j == CC - 1),
                        )
                    for j in range(CC):
                        nc.tensor.matmul(
                            pst[:, 2 * P : 3 * P],
                            lhsT=xsl(xt, i, j, 0, P),
                            rhs=zhi(j),
                            start=(j == 0),
                            stop=(j == CC - 1),
                        )
                    e = wkp.tile([P, 3 * P], BF16, name=f"e{i}", tag=f"e{i}")
                    nc.scalar.activation(e, pst[:, : 3 * P], EXP)
                    nc.gpsimd.tensor_mul(e[:, : 2 * P], e[:, : 2 * P], m2)
                    es.append(e)
                st_e[pb] = es

            def stage_b2(pb):
                # output matmuls + softmax normalization + paired store
                es, vau = st_e[pb], st_v[pb]
                st_xt[pb] = st_z2[pb] = st_v[pb] = st_e[pb] = None

                o = iop.tile([P, OW], BF16, name="o", tag="o")
                rec = wkp.tile([P, 4], F32, name="rec", tag="rec")
                for i in range(2):
                    e = es[i]
                    v0 = vau[:, (2 * i) * VW : (2 * i) * VW + VW]
                    v1 = vau[:, (2 * i + 1) * VW : (2 * i + 1) * VW + VW]

                    po0 = psp.tile([P, 512], F32, name="po0", tag="pm")[:, :VW]
                    nc.tensor.matmul(po0, lhsT=e[:, 0:P], rhs=v0, start=True, stop=True)
                    r0 = rec[:, 2 * i : 2 * i + 1]
                    nc.vector.reciprocal(r0, po0[:, H : H + 1])
                    nc.vector.tensor_scalar_mul(
                        o[:, i * 2 * H : i * 2 * H + H], po0[:, :H], r0
                    )

                    po1 = psp.tile([P, 512], F32, name="po1", tag="pm")[:, :VW]
                    nc.tensor.matmul(
                        po1, lhsT=e[:, 2 * P : 3 * P], rhs=v0, start=True, stop=False
                    )
                    nc.tensor.matmul(
                        po1, lhsT=e[:, P : 2 * P], rhs=v1, start=False, stop=True
                    )
                    r1 = rec[:, 2 * i + 1 : 2 * i + 2]
                    nc.vector.reciprocal(r1, po1[:, H : H + 1])
                    nc.vector.tensor_scalar_mul(
                        o[:, i * 2 * H + H : (i + 1) * 2 * H], po1[:, :H], r1
                    )
                nc.sync.dma_start(out_d[pb], o)

            # 2-pair skew: exp/mask latency of pair pb hides under pair
            # pb+2's projection matmuls
            for k in range(nb2 + 3):
                if 0 <= k - 3 < nb2:
                    stage_b2(k - 3)
                if 0 <= k - 2 < nb2:
                    stage_b1(k - 2)
                if k < nb2:
                    stage_a(k)

    nc.compile()
    return nc


_NC_CACHE = {}


def _get_nc(nb: int):
    if nb not in _NC_CACHE:
        _NC_CACHE[nb] = build_bass(nb)
    return _NC_CACHE[nb]


def _pack_inputs(x, Wk, Wq, Wv):
    G = ((Wq.T @ Wk) * SCALE).astype(NPBF16)  # [C, C]
    WvT = np.ascontiguousarray(Wv.T).astype(NPBF16)  # [C, H]
    tri = np.triu(np.ones((P, P), np.float32))
    M2 = np.concatenate([tri, tri], axis=1).astype(NPBF16)  # [P, 2P]
    nB = x.shape[0]
    # xt[b2, p, j*512 + i*256 + t] = x[2*b2+i, t, j*128+p]
    xt = (
        x.transpose(0, 2, 1)
        .reshape(nB // 2, 2, CC, P, T)
        .transpose(0, 3, 2, 1, 4)
        .reshape(nB // 2, P, XW)
    )
    xt = np.ascontiguousarray(xt).astype(NPBF16)
    return xt, G, WvT, M2


def _unpack_output(o, nB):
    # o[b2, p, i*768 + tc*384 + h] -> out[2*b2+i, tc*128+p, h]
    return (
        o.reshape(nB // 2, P, 2, 2, H)
        .transpose(0, 2, 3, 1, 4)
        .reshape(nB, T, H)
        .astype(np.float32)
    )


def kernel(x: np.ndarray, Wk: np.ndarray, Wq: np.ndarray, Wv: np.ndarray, **_):
    x = np.asarray(x, dtype=np.float32)
    Wk = np.asarray(Wk, dtype=np.float32)
    Wq = np.asarray(Wq, dtype=np.float32)
    Wv = np.asarray(Wv, dtype=np.float32)

    xt, G, WvT, M2 = _pack_inputs(x, Wk, Wq, Wv)
    nb = x.shape[0] // NCORES
    nc = _get_nc(nb)
    nb2 = nb // 2
    in_maps = [
        {"xt": xt[i * nb2 : (i + 1) * nb2], "G": G, "WvT": WvT, "M2": M2}
        for i in range(NCORES)
    ]
    res = run_bass_kernel_spmd(nc, in_maps, core_ids=list(range(NCORES)))
    o = np.concatenate([r["out"] for r in res.results], axis=0)
    return _unpack_output(o, x.shape[0])


if __name__ == "__main__":
    rng = np.random.default_rng(0)
    x = rng.standard_normal((B, T, C), dtype=np.float32)
    s = 1.0 / np.sqrt(C)
    Wk = rng.standard_normal((H, C), dtype=np.float32) * s
    Wq = rng.standard_normal((H, C), dtype=np.float32) * s
    Wv = rng.standard_normal((H, C), dtype=np.float32) * s
    out = kernel(x=x, Wk=Wk, Wq=Wq, Wv=Wv)
    print(out.shape, out.dtype)


# revision 15
# speedup vs baseline: 1.0171x; 1.0171x over previous
"""Single-head causal self-attention on 8 Trainium2 NeuronCores.

Reference computation (per batch b):
    k = x @ Wk.T ; q = x @ Wq.T ; v = x @ Wv.T
    wei = softmax(mask(q @ k.T / sqrt(H)))
    out = wei @ v

Strategy:
  - Data parallel: B=256 sharded across 8 cores (32 batches each), weights
    replicated. No cross-core communication.
  - Algebraic fusion: q @ k.T = x (Wq.T Wk) x.T. G = Wq.T @ Wk * scale is
    precomputed on the host, which halves the per-batch q/k projection work.
  - All per-batch operands are pre-laid-out on the host so the device never
    transposes: x is shipped as xT[b] = x[b].T (contraction dim on
    partitions), Wv as WvT = Wv.T.
  - bf16 operands (f32 PSUM accumulation): 1 cycle/row matmuls at any free
    size, half the DMA bytes, ~4.5e-3 rel err vs the 2e-2 gate.
  - Batches are processed in PAIRS: one input DMA and one output DMA per
    pair (the sync engine pays a fixed ~650ns per DMA regardless of size),
    and the z2 = G @ x.T matmuls of the two batches fuse into N=512
    instructions.
  - Scores are computed in transposed layout ST[s, t] so exp(ST) feeds the
    output matmuls as the stationary operand. Per batch the score PSUM is
    laid out [s0,t-lo | s1,t-hi | s0,t-hi] so ONE exp activation covers all
    three blocks and ONE mask multiply covers the two causal-triangle
    blocks. Causally-dead blocks (s1, t-lo) are never computed.
  - Softmax denominator: V is augmented with ones columns so the output
    matmul also yields r[t] = sum_s exp(ST[s,t]); normalization is a
    reciprocal + per-partition scale.
  - Software-pipelined schedule (A: load+projections, B1: scores+exp+mask,
    B2: output+normalize+store) with a 2-pair skew so exp/mask latency hides
    under the next pair's projection matmuls; elementwise work is spread
    over ACT/DVE/GpSimd so PE is the only near-saturated engine.
  - One merged SBUF tile per kind per pair (z2/v/e/rec) to minimize
    tile-framework semaphore count: each semaphore costs ~110ns to reset in
    the end-of-kernel teardown ladder.
"""

import ml_dtypes
import numpy as np

import concourse.bass as bass
import concourse.mybir as mybir
from concourse import bacc
import concourse.tile as tile
from concourse.bass_utils import run_bass_kernel_spmd

B, T, C, H = 256, 256, 384, 384
NCORES = 8
NB = B // NCORES  # batches per core
P = 128
CC = C // P  # 3 chunks of the embedding dim
SCALE = float(H) ** -0.5
F32 = mybir.dt.float32
BF16 = mybir.dt.bfloat16
NPBF16 = ml_dtypes.bfloat16
VW = H + 8  # v block augmented with 8 ones columns (16B-aligned in bf16)
XW = CC * 2 * T  # paired input tile width: col = j*512 + i*256 + t
OW = 2 * 2 * H  # paired output tile width: col = i*768 + tc*384 + h
VBUFS = 3  # ring depth of the merged v tile (ones written on first pass)


def build_bass(nb: int = NB):
    assert nb % 2 == 0
    nb2 = nb // 2
    nc = bacc.Bacc(
        "TRN2",
        target_bir_lowering=False,
        debug=False,
        enable_asserts=False,
        num_devices=NCORES,
    )
    xt_d = nc.dram_tensor("xt", [nb2, P, XW], BF16, kind="ExternalInput").ap()
    g_d = nc.dram_tensor("G", [C, C], BF16, kind="ExternalInput").ap()
    wvT_d = nc.dram_tensor("WvT", [C, H], BF16, kind="ExternalInput").ap()
    m_d = nc.dram_tensor("M", [P, P], BF16, kind="ExternalInput").ap()
    out_d = nc.dram_tensor("out", [nb2, P, OW], BF16, kind="ExternalOutput").ap()

    EXP = mybir.ActivationFunctionType.Exp
    CPY = mybir.ActivationFunctionType.Copy

    with tile.TileContext(nc) as tc:
        with (
            tc.tile_pool(name="const", bufs=1) as cpool,
            tc.tile_pool(name="io", bufs=3) as iop,
            tc.tile_pool(name="wk", bufs=3) as wkp,
            tc.tile_pool(name="ps", bufs=7, space="PSUM") as psp,
        ):
            # per-pair state threaded between pipeline stages
            st_xt = [None] * nb2
            st_z2 = [None] * nb2
            st_v = [None] * nb2
            st_e = [None] * nb2

            def xsl(xt, i, j, lo, hi):
                # lhsT slice of batch i, embed chunk j, seq cols [lo:hi)
                base = j * 2 * T + i * T
                return xt[:, base + lo : base + hi]

            def load_xt(pb):
                xt = iop.tile([P, XW], BF16, name="xt", tag="xt")
                nc.sync.dma_start(xt, xt_d[pb])
                st_xt[pb] = xt

            # xt[0] first on the sync queue — its ~1.3us transfer overlaps
            # the g-chunk issue; wvT/M issue in parallel from the gpsimd
            # queue
            load_xt(0)
            g_s = []
            for i in range(CC):
                gt = cpool.tile([P, C], BF16, name=f"g{i}")
                nc.sync.dma_start(gt, g_d[i * P : (i + 1) * P, :])
                g_s.append(gt)
            wvT_s = []
            for i in range(CC):
                wt = cpool.tile([P, H], BF16, name=f"wvT{i}")
                nc.gpsimd.dma_start(wt, wvT_d[i * P : (i + 1) * P, :])
                wvT_s.append(wt)
            mtri = cpool.tile([P, P], BF16, name="mtri")
            nc.gpsimd.dma_start(mtri, m_d)

            def stage_a(pb):
                # paired load + fused z2 matmuls + per-batch v projections
                if st_xt[pb] is None:
                    load_xt(pb)
                xt = st_xt[pb]

                z2 = wkp.tile([P, CC * 512], BF16, name="z2", tag="z2")
                for c2 in range(CC):
                    pz = psp.tile([P, 512], F32, name="pz", tag="pm")
                    for c1 in range(CC):
                        nc.tensor.matmul(
                            pz,
                            lhsT=g_s[c1][:, c2 * P : (c2 + 1) * P],
                            rhs=xt[:, c1 * 512 : (c1 + 1) * 512],
                            start=(c1 == 0),
                            stop=(c1 == CC - 1),
                        )
                    nc.scalar.activation(z2[:, c2 * 512 : (c2 + 1) * 512], pz, CPY)
                st_z2[pb] = z2

                vau = wkp.tile([P, 4 * VW], BF16, name="v", tag="v", bufs=VBUFS)
                if pb < VBUFS:
                    for q in range(4):
                        nc.gpsimd.memset(vau[:, q * VW + H : (q + 1) * VW], 1.0)
                for i in range(2):
                    for sc in range(2):
                        pv = psp.tile([P, 512], F32, name="pv", tag="pm")[:, :H]
                        for j in range(CC):
                            nc.tensor.matmul(
                                pv,
                                lhsT=xsl(xt, i, j, sc * P, (sc + 1) * P),
                                rhs=wvT_s[j],
                                start=(j == 0),
                                stop=(j == CC - 1),
                            )
                        q = 2 * i + sc
                        dst = vau[:, q * VW : q * VW + H]
                        if sc == 0:
                            nc.scalar.activation(dst, pv, CPY)
                        else:
                            nc.vector.tensor_copy(dst, pv)
                st_v[pb] = vau

            def stage_b1(pb):
                # scores (transposed) + single exp + single causal mask per
                # batch; psum layout [s0,t-lo | s1,t-hi | s0,t-hi]
                xt, z2 = st_xt[pb], st_z2[pb]
                es = []
                for i in range(2):
                    # psum layout: [0:256) = s0 x t[0:256), [256:384) = s1 x
                    # t[128:256) -> one exp, two triangle mask-muls
                    pst = psp.tile([P, 512], F32, name="pst", tag="pm")
                    for j in range(CC):
                        nc.tensor.matmul(
                            pst[:, 0 : 2 * P],
                            lhsT=xsl(xt, i, j, 0, P),
                            rhs=z2[:, j * 512 + i * T : j * 512 + (i + 1) * T],
                            start=(j == 0),
                            stop=(j == CC - 1),
                        )
                    for j in range(CC):
                        nc.tensor.matmul(
                            pst[:, 2 * P : 3 * P],
                            lhsT=xsl(xt, i, j, P, 2 * P),
                            rhs=z2[:, j * 512 + i * T + P : j * 512 + (i + 1) * T],
                            start=(j == 0),
                            stop=(j == CC - 1),
                        )
                    e = wkp.tile([P, 3 * P], BF16, name=f"e{i}", tag=f"e{i}")
                    nc.scalar.activation(e, pst[:, : 3 * P], EXP)
                    nc.gpsimd.tensor_mul(e[:, :P], e[:, :P], mtri)
                    nc.gpsimd.tensor_mul(e[:, 2 * P :], e[:, 2 * P :], mtri)
                    es.append(e)
                st_e[pb] = es

            def stage_b2(pb):
                # output matmuls + softmax normalization + paired store
                es, vau = st_e[pb], st_v[pb]
                st_xt[pb] = st_z2[pb] = st_v[pb] = st_e[pb] = None

                o = iop.tile([P, OW], BF16, name="o", tag="o")
                rec = wkp.tile([P, 4], F32, name="rec", tag="rec")
                for i in range(2):
                    e = es[i]
                    v0 = vau[:, (2 * i) * VW : (2 * i) * VW + VW]
                    v1 = vau[:, (2 * i + 1) * VW : (2 * i + 1) * VW + VW]

                    po0 = psp.tile([P, 512], F32, name="po0", tag="pm")[:, :VW]
                    nc.tensor.matmul(po0, lhsT=e[:, 0:P], rhs=v0, start=True, stop=True)
                    r0 = rec[:, 2 * i : 2 * i + 1]
                    nc.vector.reciprocal(r0, po0[:, H : H + 1])
                    nc.vector.tensor_scalar_mul(
                        o[:, i * 2 * H : i * 2 * H + H], po0[:, :H], r0
                    )

                    po1 = psp.tile([P, 512], F32, name="po1", tag="pm")[:, :VW]
                    nc.tensor.matmul(
                        po1, lhsT=e[:, P : 2 * P], rhs=v0, start=True, stop=False
                    )
                    nc.tensor.matmul(
                        po1, lhsT=e[:, 2 * P : 3 * P], rhs=v1, start=False, stop=True
                    )
                    r1 = rec[:, 2 * i + 1 : 2 * i + 2]
                    nc.vector.reciprocal(r1, po1[:, H : H + 1])
                    nc.vector.tensor_scalar_mul(
                        o[:, i * 2 * H + H : (i + 1) * 2 * H], po1[:, :H], r1
                    )
                nc.sync.dma_start(out_d[pb], o)

            # 2-pair skew: exp/mask latency of pair pb hides under pair
            # pb+2's projection matmuls
            for k in range(nb2 + 3):
                if 0 <= k - 3 < nb2:
                    stage_b2(k - 3)
                if 0 <= k - 2 < nb2:
                    stage_b1(k - 2)
                if k < nb2:
                    stage_a(k)

    nc.compile()
    return nc


_NC_CACHE = {}


def _get_nc(nb: int):
    if nb not in _NC_CACHE:
        _NC_CACHE[nb] = build_bass(nb)
    return _NC_CACHE[nb]


def _pack_inputs(x, Wk, Wq, Wv):
    G = ((Wq.T @ Wk) * SCALE).astype(NPBF16)  # [C, C]
    WvT = np.ascontiguousarray(Wv.T).astype(NPBF16)  # [C, H]
    M = np.triu(np.ones((P, P), np.float32)).astype(NPBF16)  # [P, P]
    nB = x.shape[0]
    # xt[b2, p, j*512 + i*256 + t] = x[2*b2+i, t, j*128+p]
    xt = (
        x.transpose(0, 2, 1)
        .reshape(nB // 2, 2, CC, P, T)
        .transpose(0, 3, 2, 1, 4)
        .reshape(nB // 2, P, XW)
    )
    xt = np.ascontiguousarray(xt).astype(NPBF16)
    return xt, G, WvT, M


def _unpack_output(o, nB):
    # o[b2, p, i*768 + tc*384 + h] -> out[2*b2+i, tc*128+p, h]
    return (
        o.reshape(nB // 2, P, 2, 2, H)
        .transpose(0, 2, 3, 1, 4)
        .reshape(nB, T, H)
        .astype(np.float32)
    )


def kernel(x: np.ndarray, Wk: np.ndarray, Wq: np.ndarray, Wv: np.ndarray, **_):
    x = np.asarray(x, dtype=np.float32)
    Wk = np.asarray(Wk, dtype=np.float32)
    Wq = np.asarray(Wq, dtype=np.float32)
    Wv = np.asarray(Wv, dtype=np.float32)

    xt, G, WvT, M = _pack_inputs(x, Wk, Wq, Wv)
    nb = x.shape[0] // NCORES
    nc = _get_nc(nb)
    nb2 = nb // 2
    in_maps = [
        {"xt": xt[i * nb2 : (i + 1) * nb2], "G": G, "WvT": WvT, "M": M}
        for i in range(NCORES)
    ]
    res = run_bass_kernel_spmd(nc, in_maps, core_ids=list(range(NCORES)))
    o = np.concatenate([r["out"] for r in res.results], axis=0)
    return _unpack_output(o, x.shape[0])


if __name__ == "__main__":
    rng = np.random.default_rng(0)
    x = rng.standard_normal((B, T, C), dtype=np.float32)
    s = 1.0 / np.sqrt(C)
    Wk = rng.standard_normal((H, C), dtype=np.float32) * s
    Wq = rng.standard_normal((H, C), dtype=np.float32) * s
    Wv = rng.standard_normal((H, C), dtype=np.float32) * s
    out = kernel(x=x, Wk=Wk, Wq=Wq, Wv=Wv)
    print(out.shape, out.dtype)
